# revision 22
# baseline (speedup 1.0000x reference)
"""Trainium2 Bass/Tile kernel for DiagnosticAttention (B=2,L=2048,H=1024,NH=16).

Sharding: 8 cores = 2 batches (data-parallel) x 4 head-blocks (tensor-parallel,
4 heads each).  Per core: Q^T/K^T projections in head-transposed layout; V with
error-gate columns and a softmax ones-column folded into the same matmul;
attention in S^T layout (keys on partitions, so diag_bias + attention_mask +
emask*sigmoid(gate) collapse into the per-partition bias of one ScalarE exp,
1024-wide tiles, and softmax denominators fall out as row 64 of (PV)^T).

v2 schedule: the ScalarE exp stream (128 x 1.34us) and the PE matmul stream
are the two near-equal resource floors; everything else is emitted so both
stay saturated.  PE warm-up matmuls + staged input DMAs compress the head;
V-projection tiles, the pair-1 q/k projections, and the chunk-0 out-projection
all ride as one-extra-per-exp-slot work items inside the four attention
passes; denominator chains run per (pair, chunk) via transposing DMAs (hidden)
or PE transposes (tail); out-projection packs head pairs (contraction 128) and
writes bf16 output so only chunk 1's projection + DMA remain on the tail.
~220us HW target, ~4.5e-3 rel err.
"""

import sys

for _p in ("/opt/trn_rl_repo", "/root/.axon_site/_ro/trn_rl_repo"):
    if _p not in sys.path:
        sys.path.insert(0, _p)

import numpy as np

B, L, H, NH = 2, 2048, 1024, 16
HD = H // NH            # 64
NCORES = 8
HPC = 4                 # heads per core
DPC = HPC * HD          # 256 head-dims per core
KT = H // 128           # 8 contraction tiles for projections
LT = L // 128           # 16 l tiles
CH = 512
HW_ = 65                    # per-head V block: 64 V cols + ones col
GW = HPC * HW_              # 260: start of gate cols
VW = GW + HPC               # 264 total

_RUNNER = None


def _build():
    import concourse.bass as bass
    import concourse.bacc as bacc
    import concourse.tile as tile
    from concourse import mybir
    from concourse.masks import make_identity

    F32 = mybir.dt.float32
    BF16 = mybir.dt.bfloat16
    AL = mybir.AluOpType
    AF = mybir.ActivationFunctionType

    nc = bacc.Bacc(None, target_bir_lowering=False)

    xT = nc.dram_tensor("xT", [H, L], BF16, kind="ExternalInput")
    wq = nc.dram_tensor("wq", [H, DPC], BF16, kind="ExternalInput")
    wk = nc.dram_tensor("wk", [H, DPC], BF16, kind="ExternalInput")
    wvg = nc.dram_tensor("wvg", [H, VW], BF16, kind="ExternalInput")
    wo = nc.dram_tensor("wo", [DPC, H], BF16, kind="ExternalInput")
    bq = nc.dram_tensor("bq", [DPC], F32, kind="ExternalInput")
    bk = nc.dram_tensor("bk", [DPC], F32, kind="ExternalInput")
    bvg = nc.dram_tensor("bvg", [VW], F32, kind="ExternalInput")
    emask = nc.dram_tensor("emask", [128, LT], F32, kind="ExternalInput")
    md = nc.dram_tensor("md", [128, LT * HPC], F32, kind="ExternalInput")
    out = nc.dram_tensor("out", [L, H], BF16, kind="ExternalOutput")
    rscr = nc.dram_tensor("rscr", [4, L], F32)

    SC = 1.0 / float(np.sqrt(HD))

    with tile.TileContext(nc) as tc:
        with (
            tc.tile_pool(name="persist", bufs=1) as P1,
            tc.tile_pool(name="es", bufs=16) as ES,
            tc.tile_pool(name="rb", bufs=2) as RB,
            tc.tile_pool(name="ps", bufs=2, space="PSUM") as PS,
        ):
            # ---- persistent SBUF tensors -------------------------------
            xt = [P1.tile([128, L], BF16, name=f"xt{k}") for k in range(KT)]
            wq_s = [P1.tile([128, DPC], BF16, name=f"wq{k}") for k in range(KT)]
            wk_s = [P1.tile([128, DPC], BF16, name=f"wk{k}") for k in range(KT)]
            wvg_s = [P1.tile([128, VW], BF16, name=f"wvg{k}") for k in range(KT)]
            wo_p = [P1.tile([128, H], BF16, name=f"wo{k}") for k in range(2)]
            qt = [P1.tile([128, L], BF16, name=f"qt{m}") for m in range(2)]
            kt = [P1.tile([128, L], BF16, name=f"kt{m}") for m in range(2)]
            v = [P1.tile([128, GW], BF16, name=f"v{t}") for t in range(LT)]
            bq_s = P1.tile([128, 2], F32, name="bqs")
            bk_s = P1.tile([128, 2], F32, name="bks")
            ebt = P1.tile([128, LT * HPC], F32, name="ebt")
            bvg_s = P1.tile([128, VW], F32, name="bvgs")
            em_s = P1.tile([128, LT], F32, name="ems")
            md_s = P1.tile([128, LT * HPC], F32, name="mds")
            bias_c = P1.tile([128, LT * HPC], F32, name="biasc")
            gp = P1.tile([128, LT * HPC], F32, name="gp")
            gs = P1.tile([128, LT * HPC], F32, name="gs")
            scr1 = P1.tile([128, 1], F32, name="scr1")
            ot_p = [P1.tile([128, L], F32, name=f"ot{h}") for h in range(2)]
            otb_p = [P1.tile([128, L], BF16, name=f"otb{h}") for h in range(2)]
            cs2 = P1.tile([65, 2 * L], F32, name="cs2")
            ident = P1.tile([128, 128], F32, name="ident")
            make_identity(nc, ident[:])

            # ---- PE warm-up: ~3.5us of fp32 matmuls on the identity so
            # the HAM clock-gate is released before the first projection --
            for _ in range(3):
                wps = PS.tile([128, 1024], F32, name="warm", tag="ss")
                for r in range(4):
                    nc.tensor.matmul(wps[:, 0:128], ident[:], ident[:],
                                     start=True, stop=True)

            # ---- input DMAs, split across both HWDGE queues so the x
            # slices land in ~half the serial time; q/k weights lead on
            # scalar (the exp stream starts much later), V weights follow.
            # gpsimd DMAs are software-generated descriptors (slow): none.
            for k in range(KT):
                nc.scalar.dma_start(out=wk_s[k][:], in_=wk[128 * k:128 * (k + 1), :])
            for k in range(0, KT, 2):
                nc.sync.dma_start(out=xt[k][:], in_=xT[128 * k:128 * (k + 1), :])
                nc.scalar.dma_start(
                    out=xt[k + 1][:], in_=xT[128 * (k + 1):128 * (k + 2), :])
            for k in range(KT):
                nc.scalar.dma_start(out=wq_s[k][:], in_=wq[128 * k:128 * (k + 1), :])
            nc.sync.dma_start(out=bvg_s[:], in_=bvg[None, :].to_broadcast((128, VW)))
            nc.sync.dma_start(out=em_s[:], in_=emask[:, :])
            nc.sync.dma_start(out=md_s[:], in_=md[:, :])
            for k in range(KT):
                nc.gpsimd.dma_start(out=wvg_s[k][:], in_=wvg[128 * k:128 * (k + 1), :])
            for k in range(2):
                nc.gpsimd.dma_start(out=wo_p[k][:], in_=wo[128 * k:128 * (k + 1), :])
                nc.sync.dma_start(
                    out=bq_s[:, k:k + 1], in_=bq[128 * k:128 * (k + 1)][:, None])
                nc.sync.dma_start(
                    out=bk_s[:, k:k + 1], in_=bk[128 * k:128 * (k + 1)][:, None])

            # preload the exp table set during the DMA window
            nc.scalar.activation(scr1[:], bvg_s[:, 0:1], AF.Exp)

            # ---- building blocks ---------------------------------------
            def proj_chunk(wi, m, c):
                wsrc, bsrc, dst = ((wq_s, bq_s, qt), (wk_s, bk_s, kt))[wi]
                ps = PS.tile([128, CH], F32, name="mm", tag="ss")
                for k in range(KT):
                    nc.tensor.matmul(
                        ps[:],
                        wsrc[k][:, 128 * m:128 * (m + 1)],
                        xt[k][:, CH * c:CH * (c + 1)],
                        start=(k == 0), stop=(k == KT - 1))
                nc.vector.tensor_scalar_add(
                    dst[m][:, CH * c:CH * (c + 1)], ps[:], bsrc[:, m:m + 1])

            def vtile_mm(t):
                # V matmuls + the two DVE psum readers (psum frees within
                # the slot, keeping the ss ring flowing)
                ps = PS.tile([128, CH], F32, name="vv", tag="ss")
                pv = ps[:, 0:VW]
                for k in range(KT):
                    nc.tensor.matmul(
                        pv,
                        xt[k][:, 128 * t:128 * (t + 1)],
                        wvg_s[k][:],
                        start=(k == 0), stop=(k == KT - 1))
                nc.vector.tensor_add(gp[:, HPC * t:HPC * (t + 1)],
                                     ps[:, GW:VW], bvg_s[:, GW:VW])
                # V(+bias) -> SBUF; ones columns come via bvg
                nc.vector.tensor_add(v[t][:], pv[:, 0:GW], bvg_s[:, 0:GW])

            def vtile_fin(t):
                # gate sigmoid + key bias, then exp(bias) folded into the
                # V rows (incl. the ones column, so the softmax denominator
                # picks it up too).  Emitted >=2 slots after vtile_mm so the
                # ACT queue never blocks on the V matmuls.
                g = gs[:, HPC * t:HPC * (t + 1)]
                nc.scalar.activation(g, gp[:, HPC * t:HPC * (t + 1)],
                                     AF.Exp, scale=-1.0)
                nc.vector.tensor_scalar_add(g, g, 1.0)
                nc.vector.reciprocal(g, g)
                nc.vector.scalar_tensor_tensor(
                    out=bias_c[:, HPC * t:HPC * (t + 1)],
                    in0=g, scalar=em_s[:, t:t + 1],
                    in1=md_s[:, HPC * t:HPC * (t + 1)],
                    op0=AL.mult, op1=AL.add)
                nc.scalar.activation(ebt[:, HPC * t:HPC * (t + 1)],
                                     bias_c[:, HPC * t:HPC * (t + 1)], AF.Exp)
                for h in range(HPC):
                    nc.vector.tensor_scalar_mul(
                        v[t][:, HW_ * h:HW_ * (h + 1)],
                        v[t][:, HW_ * h:HW_ * (h + 1)],
                        ebt[:, HPC * t + h:HPC * t + h + 1])

            def vtile(t):
                vtile_mm(t)
                vtile_fin(t)

            def att_pass(hp, cp, extras):
                # extras: dict slot -> list of closures, emitted after that
                # slot's PV matmuls (their deps must already be met there —
                # engine queues execute in order, so a stalled extra blocks
                # everything behind it on its engine)
                ha, hb = 2 * hp, 2 * hp + 1
                pvs = {}
                for h in (ha, hb):
                    for j in range(2):
                        pvs[(h, j)] = PS.tile(
                            [128, CH], F32, name="pv", tag="pv", bufs=4)
                for m in range(LT):
                    # per (m, j): one [128, 1024] ss tile holds BOTH heads
                    # (512 cols each).  The two S matmuls hit disjoint
                    # 64-row groups of the PE array and are emitted
                    # back-to-back, so they stream concurrently; exp(bias)
                    # is folded into the V rows (vtile_fin), so one
                    # bias-free exp covers the whole tile.
                    for j in range(2):
                        ssj = PS.tile([128, 2 * CH], F32, name="ss2",
                                      tag="ss")
                        for h in (ha, hb):
                            hf = 64 * (h % 2)
                            nc.tensor.matmul(
                                ssj[:, CH * (h % 2):CH * (h % 2 + 1)],
                                kt[hp][hf:hf + 64, 128 * m:128 * (m + 1)],
                                qt[hp][hf:hf + 64,
                                       1024 * cp + CH * j:1024 * cp + CH * (j + 1)],
                                start=True, stop=True)
                        es2 = ES.tile([128, 2 * CH], BF16, name="es")
                        nc.scalar.activation(es2[:], ssj[:], AF.Exp, scale=SC)
                        for h in (ha, hb):
                            nc.tensor.matmul(
                                pvs[(h, j)][0:HW_, :],
                                v[m][:, HW_ * h:HW_ * (h + 1)],
                                es2[:, CH * (h % 2):CH * (h % 2 + 1)],
                                start=(m == 0), stop=(m == LT - 1))
                    for fn in extras.get(m, ()):
                        fn()
                # drain: denominator row -> cs2; head rows -> packed ot
                for h in (ha, hb):
                    for j in range(2):
                        pv = pvs[(h, j)]
                        cc = 2 * cp + j
                        nc.vector.tensor_copy(
                            cs2[HD:HD + 1, L * (h % 2) + CH * cc:
                                L * (h % 2) + CH * (cc + 1)],
                            pv[HD:HD + 1, :])
                        nc.vector.tensor_copy(
                            ot_p[hp][64 * (h % 2):64 * (h % 2) + HD,
                                     CH * cc:CH * (cc + 1)],
                            pv[0:HD, :])

            def chain_fin1(hp, cp):
                # bounce the raw denominator rows through DRAM (SBUF
                # sources cannot broadcast across partitions) on the
                # otherwise idle gpsimd queue
                ha, hb = 2 * hp, 2 * hp + 1
                for hloc, h in ((0, ha), (1, hb)):
                    seg = slice(L * hloc + 1024 * cp, L * hloc + 1024 * (cp + 1))
                    nc.gpsimd.dma_start(
                        out=rscr[h, 1024 * cp:1024 * (cp + 1)][None, :],
                        in_=cs2[HD:HD + 1, seg])

            def chain_fin2(hp, cp):
                # broadcast to [128,1024] (head pair stacked 64/64), then
                # approx-reciprocal (~51 ULP, ~5x faster than the exact
                # reciprocal's ~6 cyc/elem; denominators are ~1e3, no edge
                # cases)
                ha, hb = 2 * hp, 2 * hp + 1
                rb = RB.tile([128, 1024], F32, name="rb")
                for hloc, h in ((0, ha), (1, hb)):
                    nc.gpsimd.dma_start(
                        out=rb[64 * hloc:64 * (hloc + 1), :],
                        in_=rscr[h, 1024 * cp:1024 * (cp + 1)][None, :]
                        .to_broadcast((64, 1024)))
                nc.vector.reciprocal_approx_fast(out=rb[:], in_=rb[:])
                nc.vector.tensor_mul(
                    otb_p[hp][:, 1024 * cp:1024 * (cp + 1)],
                    ot_p[hp][:, 1024 * cp:1024 * (cp + 1)], rb[:])

            def outproj_item(cp, ti, n, queue, tag="ss", copy_eng=None):
                # one [128 q, 512] out tile: both packed head-pair matmuls
                # (contraction 128), bf16 stage, DMA on the given queue.
                # PSUM comes from the fast-cycling ss ring: the pv ring is
                # held for a whole pass by the PV accumulators, so a mid-pass
                # pv-tag alloc would stall the PE queue until pass end.
                t = 8 * cp + ti
                ps = PS.tile([128, CH], F32, name="mm", tag=tag,
                             bufs=(4 if tag == "pv" else None))
                nc.tensor.matmul(
                    ps[:], otb_p[0][:, 128 * t:128 * (t + 1)],
                    wo_p[0][:, CH * n:CH * (n + 1)],
                    start=True, stop=False)
                nc.tensor.matmul(
                    ps[:], otb_p[1][:, 128 * t:128 * (t + 1)],
                    wo_p[1][:, CH * n:CH * (n + 1)],
                    start=False, stop=True)
                stage = P1.tile([128, CH], BF16, name=f"st{(2 * ti + n) % 2}")
                if copy_eng is None:
                    nc.vector.tensor_copy(stage[:], ps[:])
                else:
                    copy_eng(stage[:], ps[:])
                queue.dma_start(
                    out=out[128 * t:128 * (t + 1), CH * n:CH * (n + 1)],
                    in_=stage[:])

            # ---- schedule ----------------------------------------------
            # head: just enough for pass (pair0, chunk0) to start streaming
            proj_chunk(1, 0, 0)              # kt[0] keys 0-511
            proj_chunk(0, 0, 0)              # qt[0] queries 0-511
            proj_chunk(0, 0, 1)              # qt[0] queries 512-1023
            vtile(0)
            vtile_mm(1)
            vtile_mm(2)

            def EX(*pairs):
                d = {}
                for s, fn in pairs:
                    d.setdefault(s, []).append(fn)
                return d

            # pass p0c0: V tiles stay ~2 slots ahead of their PV matmuls;
            # each vtile_fin is emitted BEFORE slot t so PV(m=t) reads the
            # exp(bias)-scaled V rows.  kt[0] chunks land just before their
            # m-tiles; pair-1 k/q projections start at the end.
            ex0 = [(t - 3, lambda t=t: vtile_mm(t)) for t in range(3, LT)]
            ex0 += [(t - 1, lambda t=t: vtile_fin(t)) for t in range(1, LT)]
            ex0 += [(1, lambda: proj_chunk(1, 0, 1)),
                    (4, lambda: proj_chunk(1, 0, 2)),
                    (7, lambda: proj_chunk(1, 0, 3)),
                    (13, lambda: proj_chunk(1, 1, 0)),
                    (14, lambda: proj_chunk(0, 1, 0)),
                    (15, lambda: proj_chunk(0, 1, 1))]
            att_pass(0, 0, EX(*ex0))
            chain_fin1(0, 0)

            # pass p1c0: rest of projections + pair-0 chunk-0 chain pieces
            att_pass(1, 0, EX(
                (0, lambda: proj_chunk(1, 1, 1)),
                (2, lambda: proj_chunk(1, 1, 2)),
                (4, lambda: proj_chunk(1, 1, 3)),
                (4, lambda: chain_fin2(0, 0)),
                (6, lambda: proj_chunk(0, 0, 2)),
                (8, lambda: proj_chunk(0, 0, 3)),
                (10, lambda: proj_chunk(0, 1, 2)),
                (12, lambda: proj_chunk(0, 1, 3)),
            ))
            chain_fin1(1, 0)

            # pass p0c1: pair-1 chunk-0 chain, then chunk-0 out-projection
            # spread one tile per slot (sync queue: scalar is the exp stream)
            items = [(ti, n) for ti in range(8) for n in range(2)]
            ex2 = [(4, lambda: chain_fin2(1, 0))]
            for idx, (ti, n) in enumerate(items[:10]):
                ex2.append((6 + idx,
                            lambda ti=ti, n=n: outproj_item(0, ti, n, nc.sync)))
            att_pass(0, 1, EX(*ex2))
            chain_fin1(0, 1)

            # pass p1c1: chunk-0 out-projection rest + pair-0 chunk-1 chain
            ex3 = [(idx, lambda ti=ti, n=n: outproj_item(0, ti, n, nc.sync))
                   for idx, (ti, n) in enumerate(items[10:])]
            ex3 += [(8, lambda: chain_fin2(0, 1))]
            att_pass(1, 1, EX(*ex3))
            chain_fin1(1, 1)

            # tail: pair-1 chunk-1 chain, then the last 8 output tiles,
            # DMAs alternating between both HWDGE queues (exp stream done)
            # keep the PE clock warm across the chain-latency window
            for _ in range(2):
                wps = PS.tile([128, 1024], F32, name="warm", tag="ss")
                for r in range(3):
                    nc.tensor.matmul(wps[:, 0:128], ident[:], ident[:],
                                     start=True, stop=True)
            chain_fin2(1, 1)
            oq = [nc.sync, nc.scalar]
            ce = [None, nc.scalar.copy]
            for idx, (ti, n) in enumerate(items):
                outproj_item(1, ti, n, oq[idx % 2],
                             tag=("ss" if idx % 2 else "pv"),
                             copy_eng=ce[idx % 2])

    nc.finalize()
    return nc


def _make_runner():
    """Compile once; return f(in_maps) -> list of per-core output dicts.

    Same execution path as concourse.bass_utils.run_bass_kernel_spmd under
    axon (bass2jax custom-call via PJRT), but with the jitted executable
    cached so repeated calls don't recompile.
    """
    import jax
    from jax.experimental.shard_map import shard_map
    from jax.sharding import Mesh, PartitionSpec
    from concourse import bass2jax, mybir

    nc = _build()
    bass2jax.install_neuronx_cc_hook()

    partition_name = nc.partition_id_tensor.name if nc.partition_id_tensor else None
    in_names, out_names, out_avals, zero_outs = [], [], [], []
    for alloc in nc.m.functions[0].allocations:
        if not isinstance(alloc, mybir.MemoryLocationSet):
            continue
        name = alloc.memorylocations[0].name
        if alloc.kind == "ExternalInput":
            if name != partition_name:
                in_names.append(name)
        elif alloc.kind == "ExternalOutput":
            out_names.append(name)
            shape = tuple(alloc.tensor_shape)
            dtype = mybir.dt.np(alloc.dtype)
            out_avals.append(jax.core.ShapedArray(shape, dtype))
            zero_outs.append(np.zeros(shape, dtype))
    n_params = len(in_names)
    n_outs = len(out_avals)
    feed_names = list(in_names) + list(out_names)
    if partition_name is not None:
        feed_names.append(partition_name)
    donate = tuple(range(n_params, n_params + n_outs))

    def _body(*args):
        operands = list(args)
        if partition_name is not None:
            operands.append(bass2jax.partition_id_tensor())
        outs = bass2jax._bass_exec_p.bind(
            *operands,
            out_avals=tuple(out_avals),
            in_names=tuple(feed_names),
            out_names=tuple(out_names),
            lowering_input_output_aliases=(),
            sim_require_finite=True,
            sim_require_nnan=True,
            nc=nc,
        )
        return tuple(outs)

    devices = jax.devices()[:NCORES]
    mesh = Mesh(np.asarray(devices), ("core",))
    sharded = jax.jit(
        shard_map(
            _body, mesh=mesh,
            in_specs=(PartitionSpec("core"),) * (n_params + n_outs),
            out_specs=(PartitionSpec("core"),) * n_outs,
            check_rep=False,
        ),
        donate_argnums=donate, keep_unused=True,
    )

    def run(in_maps):
        gi = [np.concatenate([np.asarray(m[nm]) for m in in_maps], axis=0)
              for nm in in_names]
        go = [np.concatenate([z] * NCORES, axis=0) for z in zero_outs]
        outs = sharded(*gi, *go)
        res = []
        for i in range(NCORES):
            d = {}
            for j, nm in enumerate(out_names):
                n0 = zero_outs[j].shape[0]
                d[nm] = np.asarray(outs[j][i * n0:(i + 1) * n0])
            res.append(d)
        return res

    from jax.sharding import NamedSharding
    shd = NamedSharding(mesh, PartitionSpec("core"))
    gshapes = [(NCORES * z.shape[0],) + z.shape[1:] for z in zero_outs]
    gdtypes = [z.dtype for z in zero_outs]
    make_zeros = jax.jit(
        lambda: tuple(
            jax.numpy.zeros(s, d) for s, d in zip(gshapes, gdtypes)),
        out_shardings=(shd,) * n_outs)

    def run_timed(in_maps, iters=10):
        """Device-resident repeat timing: returns list of per-iter seconds."""
        import time
        gi = [jax.device_put(
            np.concatenate([np.asarray(m[nm]) for m in in_maps], axis=0), shd)
            for nm in in_names]
        jax.block_until_ready(gi)
        ts = []
        for _ in range(iters):
            go = make_zeros()
            jax.block_until_ready(go)
            t0 = time.perf_counter()
            outs = sharded(*gi, *go)
            jax.block_until_ready(outs)
            ts.append(time.perf_counter() - t0)
        return ts

    run.timed = run_timed
    return run


def _shard_inputs(hidden_states, attention_mask, has_error_codes,
                  Wq, bq, Wk, bk, Wv, bv, Wo, bo, diag_bias, Wg, bg):
    import ml_dtypes
    bf16 = ml_dtypes.bfloat16
    f32 = np.float32
    hs = np.asarray(hidden_states, f32)
    am = np.asarray(attention_mask, f32).reshape(B, L)
    ec = np.asarray(has_error_codes).astype(f32)
    Wq, Wk, Wv, Wo = (np.asarray(w, f32) for w in (Wq, Wk, Wv, Wo))
    Wg = np.asarray(Wg, f32)
    bq, bk, bv, bg = (np.asarray(x, f32) for x in (bq, bk, bv, bg))
    diag = np.asarray(diag_bias, f32).reshape(NH)

    in_maps = []
    for core in range(NCORES):
        b, hb = core // 4, core % 4
        heads = range(4 * hb, 4 * hb + 4)
        cols = slice(DPC * hb, DPC * (hb + 1))
        wvg = np.zeros((H, VW), f32)
        bvg = np.zeros((VW,), f32)
        for j, h in enumerate(heads):
            wvg[:, HW_ * j:HW_ * j + HD] = Wv[:, HD * h:HD * (h + 1)]
            bvg[HW_ * j:HW_ * j + HD] = bv[HD * h:HD * (h + 1)]
            wvg[:, GW + j] = Wg[:, h]
            bvg[GW + j] = bg[h]
            bvg[HW_ * j + HD] = 1.0
        mdv = am[b][:, None] + diag[list(heads)][None, :]          # (L, 4)
        in_maps.append({
            "xT": np.ascontiguousarray(hs[b].T).astype(bf16),
            "wq": np.ascontiguousarray(Wq[:, cols]).astype(bf16),
            "wk": np.ascontiguousarray(Wk[:, cols]).astype(bf16),
            "wvg": wvg.astype(bf16),
            "wo": np.ascontiguousarray(Wo[cols, :]).astype(bf16),
            "bq": np.ascontiguousarray(bq[cols]),
            "bk": np.ascontiguousarray(bk[cols]),
            "bvg": bvg,
            "emask": np.ascontiguousarray(ec[b].reshape(LT, 128).T),
            "md": np.ascontiguousarray(
                mdv.reshape(LT, 128, HPC).transpose(1, 0, 2).reshape(128, LT * HPC)),
        })
    return in_maps


def kernel(**inputs) -> np.ndarray:
    global _RUNNER
    if _RUNNER is None:
        _RUNNER = _make_runner()
    in_maps = _shard_inputs(**inputs)
    results = _RUNNER(in_maps)
    bo = np.asarray(inputs["bo"], np.float32)
    out = np.zeros((B, L, H), np.float32)
    for b in range(B):
        acc = np.zeros((L, H), np.float64)
        for j in range(4):
            acc += results[4 * b + j]["out"].astype(np.float64)
        out[b] = (acc + bo.astype(np.float64)).astype(np.float32)
    return out
